# revision 2
# baseline (speedup 1.0000x reference)
"""Adaptive embedding lookup (3 vocab clusters + projections) on 8 TRN2 cores.

Strategy: data-parallel over batch. Each of the 8 NeuronCores gets one
batch row (4096 tokens) plus a full replica of the (small) embedding
tables and projection matrices; there are no collectives. Per 128-token
tile the kernel:
  1. indirect-DMA gathers the token rows from all three tables
     (out-of-cluster tokens gather a clamped row and are masked to 0),
  2. projects the 256-d and 64-d rows to 1024 with PE matmuls
     (PE transpose of the gathered tile feeds lhsT),
  3. fuses mask*scale of the 1024-d cluster with the PSUM accumulation,
  4. streams the [128, 1024] result tile back to HBM.
"""

import os

import numpy as np

import concourse.bass as bass
import concourse.tile as tile
from concourse import bacc, mybir
from concourse.bass import IndirectOffsetOnAxis
from concourse.masks import make_identity

P = 128
D = 1024
V0, V1, V2 = 20000, 40000, 68000
C0, C1 = 20000, 60000
E1, E2 = 256, 64
SCALE = 32.0  # sqrt(D)
F32 = mybir.dt.float32
I32 = mybir.dt.int32
ALU = mybir.AluOpType

N_CORES = 8
S_FULL = 4096  # tokens per core (one batch row)

# set by kernel() when profiling is enabled via KERNEL_PROFILE=1
last_exec_time_ns = None


def build(S=S_FULL, TB=1):
    """Build the single-core Bass graph (same program on all 8 cores)."""
    NT = S // P
    NB = NT // TB
    assert NT % TB == 0

    nc = bacc.Bacc("TRN2", target_bir_lowering=False, debug=False,
                   num_devices=N_CORES)
    ids = nc.dram_tensor("ids", [S], I32, kind="ExternalInput").ap()
    emb0 = nc.dram_tensor("emb0", [V0, D], F32, kind="ExternalInput").ap()
    emb1 = nc.dram_tensor("emb1", [V1, E1], F32, kind="ExternalInput").ap()
    emb2 = nc.dram_tensor("emb2", [V2, E2], F32, kind="ExternalInput").ap()
    # proj{1,2}.T pre-scaled by sqrt(D), shapes [E, D]
    p1t = nc.dram_tensor("p1t", [E1, D], F32, kind="ExternalInput").ap()
    p2t = nc.dram_tensor("p2t", [E2, D], F32, kind="ExternalInput").ap()
    out = nc.dram_tensor("out", [S, D], F32, kind="ExternalOutput").ap()

    # token (p, t) = p*NT + t: contiguous ids per partition
    ids_r = ids.rearrange("(p t) -> p t", t=NT)
    out_r = out.rearrange("(p t) d -> p t d", t=NT)

    with tile.TileContext(nc) as tc:
        with (
            tc.tile_pool(name="const", bufs=1) as cpool,
            tc.tile_pool(name="gather", bufs=2) as gpool,
            tc.tile_pool(name="work", bufs=3) as wpool,
            tc.tile_pool(name="lhs", bufs=3) as lpool,
            tc.tile_pool(name="outp", bufs=3) as opool,
            tc.tile_pool(name="pmm", bufs=2, space="PSUM") as pmm,
            tc.tile_pool(name="ptr", bufs=1, space="PSUM") as ptr,
        ):
            ident = cpool.tile([P, P], F32)
            make_identity(nc, ident[:])

            # projection weights: p1t as two K-chunks side by side
            p1t_sb = cpool.tile([P, 2 * D], F32)
            nc.sync.dma_start(out=p1t_sb[:, 0:D], in_=p1t[0:P, :])
            nc.sync.dma_start(out=p1t_sb[:, D:2 * D], in_=p1t[P:2 * P, :])
            p2t_sb = cpool.tile([E2, D], F32)
            nc.sync.dma_start(out=p2t_sb[:], in_=p2t[:, :])

            ids_sb = cpool.tile([P, NT], I32)
            nc.sync.dma_start(out=ids_sb[:], in_=ids_r)
            ids_f = cpool.tile([P, NT], F32)
            nc.vector.tensor_copy(ids_f[:], ids_sb[:])

            # masks: 0/1 step functions of the id
            ge1 = cpool.tile([P, NT], F32)
            nc.vector.tensor_scalar(out=ge1[:], in0=ids_f[:], scalar1=0.5,
                                    scalar2=None, op0=ALU.is_ge)
            ge20 = cpool.tile([P, NT], F32)
            nc.vector.tensor_scalar(out=ge20[:], in0=ids_f[:], scalar1=C0 - 0.5,
                                    scalar2=None, op0=ALU.is_ge)
            ge60 = cpool.tile([P, NT], F32)
            nc.vector.tensor_scalar(out=ge60[:], in0=ids_f[:], scalar1=C1 - 0.5,
                                    scalar2=None, op0=ALU.is_ge)
            m0v = cpool.tile([P, NT], F32)  # SCALE * (1 <= id < C0)
            nc.vector.tensor_tensor(out=m0v[:], in0=ge1[:], in1=ge20[:],
                                    op=ALU.subtract)
            nc.vector.tensor_scalar_mul(out=m0v[:], in0=m0v[:], scalar1=SCALE)
            m1v = cpool.tile([P, NT], F32)  # (C0 <= id < C1)
            nc.vector.tensor_tensor(out=m1v[:], in0=ge20[:], in1=ge60[:],
                                    op=ALU.subtract)
            m2v = ge60  # (C1 <= id)

            # clamped local row ids per cluster (int32)
            lidf = cpool.tile([P, NT], F32)
            lid0 = cpool.tile([P, NT], I32)
            nc.vector.tensor_scalar(out=lidf[:], in0=ids_f[:],
                                    scalar1=float(V0 - 1), scalar2=None,
                                    op0=ALU.min)
            nc.vector.tensor_copy(lid0[:], lidf[:])
            lid1 = cpool.tile([P, NT], I32)
            nc.vector.tensor_scalar(out=lidf[:], in0=ids_f[:],
                                    scalar1=float(C0), op0=ALU.max,
                                    scalar2=float(C0 + V1 - 1), op1=ALU.min)
            nc.vector.tensor_scalar(out=lidf[:], in0=lidf[:],
                                    scalar1=float(C0), scalar2=None,
                                    op0=ALU.subtract)
            nc.vector.tensor_copy(lid1[:], lidf[:])
            lid2 = cpool.tile([P, NT], I32)
            nc.vector.tensor_scalar(out=lidf[:], in0=ids_f[:],
                                    scalar1=float(C1), op0=ALU.max,
                                    scalar2=float(C1 + V2 - 1), op1=ALU.min)
            nc.vector.tensor_scalar(out=lidf[:], in0=lidf[:],
                                    scalar1=float(C1), scalar2=None,
                                    op0=ALU.subtract)
            nc.vector.tensor_copy(lid2[:], lidf[:])

            for bt in range(NB):
                sl = slice(bt * TB, (bt + 1) * TB)
                g0b = gpool.tile([P, TB * D], F32)
                nc.gpsimd.indirect_dma_start(
                    out=g0b[:], out_offset=None, in_=emb0[:, :],
                    in_offset=IndirectOffsetOnAxis(ap=lid0[:, sl], axis=0))
                g1b = gpool.tile([P, TB * E1], F32)
                nc.gpsimd.indirect_dma_start(
                    out=g1b[:], out_offset=None, in_=emb1[:, :],
                    in_offset=IndirectOffsetOnAxis(ap=lid1[:, sl], axis=0))
                g2b = gpool.tile([P, TB * E2], F32)
                nc.gpsimd.indirect_dma_start(
                    out=g2b[:], out_offset=None, in_=emb2[:, :],
                    in_offset=IndirectOffsetOnAxis(ap=lid2[:, sl], axis=0))

                for j in range(TB):
                    t = bt * TB + j
                    tcol = slice(t, t + 1)
                    g1m = wpool.tile([P, E1], F32)
                    nc.vector.tensor_scalar_mul(
                        out=g1m[:], in0=g1b[:, j * E1:(j + 1) * E1],
                        scalar1=m1v[:, tcol])
                    g2m = wpool.tile([P, E2], F32)
                    nc.vector.tensor_scalar_mul(
                        out=g2m[:], in0=g2b[:, j * E2:(j + 1) * E2],
                        scalar1=m2v[:, tcol])

                    tA = ptr.tile([P, P], F32, tag="tA")
                    nc.tensor.transpose(out=tA[:], in_=g1m[:, 0:P],
                                        identity=ident[:])
                    tB = ptr.tile([P, P], F32, tag="tB")
                    nc.tensor.transpose(out=tB[:], in_=g1m[:, P:2 * P],
                                        identity=ident[:])
                    tC = ptr.tile([E2, P], F32, tag="tC")
                    nc.tensor.transpose(out=tC[:], in_=g2m[:],
                                        identity=ident[:])

                    lhs1 = lpool.tile([P, 2 * P], F32)
                    nc.scalar.copy(out=lhs1[:, 0:P], in_=tA[:])
                    nc.scalar.copy(out=lhs1[:, P:2 * P], in_=tB[:])
                    lhs2 = lpool.tile([E2, P], F32)
                    nc.scalar.copy(out=lhs2[:], in_=tC[:])

                    po = pmm.tile([P, D], F32)
                    for n in range(2):
                        ns = slice(n * 512, (n + 1) * 512)
                        nc.tensor.matmul(out=po[:, ns], lhsT=lhs1[:, 0:P],
                                         rhs=p1t_sb[:, n * 512:(n + 1) * 512],
                                         start=True, stop=False)
                        nc.tensor.matmul(out=po[:, ns], lhsT=lhs1[:, P:2 * P],
                                         rhs=p1t_sb[:, D + n * 512:D + (n + 1) * 512],
                                         start=False, stop=False)
                        nc.tensor.matmul(out=po[:, ns], lhsT=lhs2[:],
                                         rhs=p2t_sb[:, ns],
                                         start=False, stop=True)

                    ot = opool.tile([P, D], F32)
                    for n in range(2):
                        ns = slice(n * 512, (n + 1) * 512)
                        nc.vector.scalar_tensor_tensor(
                            out=ot[:, ns],
                            in0=g0b[:, j * D + n * 512:j * D + (n + 1) * 512],
                            scalar=m0v[:, tcol], in1=po[:, ns],
                            op0=ALU.mult, op1=ALU.add)
                    nc.sync.dma_start(out=out_r[:, t, :], in_=ot[:])

    nc.compile()
    return nc


def _prep_host_inputs(input_ids, emb0, emb1, emb2, proj1, proj2):
    ids = np.ascontiguousarray(np.asarray(input_ids, dtype=np.int32))
    emb0 = np.ascontiguousarray(np.asarray(emb0, dtype=np.float32))
    emb1 = np.ascontiguousarray(np.asarray(emb1, dtype=np.float32))
    emb2 = np.ascontiguousarray(np.asarray(emb2, dtype=np.float32))
    p1t = np.ascontiguousarray(np.asarray(proj1, dtype=np.float32).T * SCALE)
    p2t = np.ascontiguousarray(np.asarray(proj2, dtype=np.float32).T * SCALE)
    return ids, emb0, emb1, emb2, p1t, p2t


def kernel(input_ids, emb0, emb1, emb2, proj1, proj2):
    global last_exec_time_ns
    from concourse.bass_utils import run_bass_kernel_spmd

    ids, emb0, emb1, emb2, p1t, p2t = _prep_host_inputs(
        input_ids, emb0, emb1, emb2, proj1, proj2)
    B, S = ids.shape
    assert B == N_CORES and S == S_FULL, (B, S)

    nc = build(S)

    # token (p, t) = p*NT + t per core: pass ids reordered to match the
    # device's [P, NT] view being a plain reshape of the DRAM buffer.
    in_maps = []
    for b in range(B):
        in_maps.append({
            "ids": np.ascontiguousarray(ids[b]),
            "emb0": emb0, "emb1": emb1, "emb2": emb2,
            "p1t": p1t, "p2t": p2t,
        })

    profile = os.environ.get("KERNEL_PROFILE", "0") == "1"
    res = run_bass_kernel_spmd(nc, in_maps, core_ids=list(range(N_CORES)),
                               trace=profile)
    last_exec_time_ns = res.exec_time_ns
    out = np.stack([res.results[b]["out"] for b in range(B)], axis=0)
    return out
